# revision 4
# baseline (speedup 1.0000x reference)
"""Cross-attention kernel for 8 Trainium2 NeuronCores.

Problem (hardcoded): B=2, NQ=NKV=2048, QDIM=KVDIM=1024, H=16, HD=64.

Sharding: tensor-parallel over heads — 2 heads per core. Each core computes
its heads' Q/K/V projections, scores, softmax and context for the full
sequence, then an AllToAll reshards context from head-split to token-split
so the output projection is fully local; core j returns output tokens
[j*512, (j+1)*512).

All matmuls run in bf16 (fp32 PSUM accumulation). Layout trick: inputs are
fed pre-transposed ([feature, token]) so every matmul operand already has
its contraction dim on partitions — the kernel contains zero on-device
transposes. scores are computed transposed ([k, q]) so the exp'd
probabilities feed the P@V matmul directly as the stationary operand, and a
ones-column appended to V yields the softmax denominator from the same
matmul (no partition-axis reduction needed).
"""

import numpy as np
import ml_dtypes

import concourse.bass as bass
import concourse.mybir as mybir
import concourse.tile as tile
from concourse import bacc
from concourse.bass_utils import run_bass_kernel_spmd

N_CORES = 8
B = 2
NQ = NKV = 2048
C = 1024          # model dim (QDIM=KVDIM=INNER)
H, HD = 16, 64
T = B * NQ        # 4096 flattened tokens
DL = 128          # local head dims per core (2 heads * 64)
TSH = T // N_CORES  # 512 output tokens per core
SCALE = HD ** -0.5

F32 = mybir.dt.float32
BF16 = mybir.dt.bfloat16

_NC_CACHE = None
_LAST_RESULTS = None


def _build(with_collective=True):
    nc = bacc.Bacc("TRN2", target_bir_lowering=False, debug=False,
                   num_devices=N_CORES)

    qT = nc.dram_tensor("qT", [C, T], BF16, kind="ExternalInput")
    kvT = nc.dram_tensor("kvT", [C, T], BF16, kind="ExternalInput")
    wq = nc.dram_tensor("wq", [C, DL], BF16, kind="ExternalInput")
    wk = nc.dram_tensor("wk", [C, DL], BF16, kind="ExternalInput")
    wv = nc.dram_tensor("wv", [C, DL], BF16, kind="ExternalInput")
    wo = nc.dram_tensor("wo", [C, C], BF16, kind="ExternalInput")
    bias = nc.dram_tensor("bias", [C], F32, kind="ExternalInput")
    out = nc.dram_tensor("out", [TSH, C], F32, kind="ExternalOutput")

    CC = C // 128   # 8 contraction chunks
    KT = NKV // 128  # 16 k-tiles per batch
    Exp = mybir.ActivationFunctionType.Exp

    with tile.TileContext(nc) as tc:
        with (
            tc.tile_pool(name="consts", bufs=1) as consts,
            tc.tile_pool(name="xt", bufs=2) as xt,
            tc.tile_pool(name="probs", bufs=4) as probs_p,
            tc.tile_pool(name="norm", bufs=2) as norm,
            tc.tile_pool(name="outp", bufs=2) as outp,
            tc.tile_pool(name="dram", bufs=1, space="DRAM") as dram,
        ):
            # ---- constants ----
            wq_sb = consts.tile([128, CC, DL], BF16)
            nc.sync.dma_start(out=wq_sb, in_=wq.ap().rearrange("(n p) d -> p n d", p=128))
            wk_sb = consts.tile([128, CC, DL], BF16)
            nc.sync.dma_start(out=wk_sb, in_=wk.ap().rearrange("(n p) d -> p n d", p=128))
            wv_sb = consts.tile([128, CC, DL], BF16)
            nc.sync.dma_start(out=wv_sb, in_=wv.ap().rearrange("(n p) d -> p n d", p=128))
            wo_sb = consts.tile([128, CC, C], BF16)
            nc.sync.dma_start(out=wo_sb, in_=wo.ap().rearrange("(n p) e -> p n e", p=128))
            bias_sb = consts.tile([128, C], F32)
            bias_bc = bass.AP(tensor=bias, offset=0, ap=[[0, 128], [1, C]])
            nc.gpsimd.dma_start(out=bias_sb[:], in_=bias_bc)

            # persistent activations
            Kd_sb = consts.tile([128, T], BF16)   # K^T: [d_local, token]
            Qd_sb = consts.tile([128, T], BF16)   # Q^T: [d_local, token]
            # V natural [token, d] in 32 tiles of [128, 130]:
            # cols 0:64 = head0, col 64 = ones, 65:129 = head1, col 129 = ones
            V_sb = consts.tile([128, T // 128, 130], BF16)
            nc.vector.memset(V_sb[:, :, 64:65], 1.0)
            nc.vector.memset(V_sb[:, :, 129:130], 1.0)

            qT_r = qT.ap().rearrange("(n p) t -> p n t", p=128)
            kvT_r = kvT.ap().rearrange("(n p) t -> p n t", p=128)

            # ---- projections ----
            with tc.tile_pool(name="psP", bufs=2, space="PSUM") as psP:
                for tt in range(T // 512):
                    t0 = tt * 512
                    kvt = xt.tile([128, CC, 512], BF16, tag="kvt")
                    nc.sync.dma_start(out=kvt, in_=kvT_r[:, :, t0:t0 + 512])
                    qt_ = xt.tile([128, CC, 512], BF16, tag="qt")
                    nc.sync.dma_start(out=qt_, in_=qT_r[:, :, t0:t0 + 512])

                    psk = psP.tile([128, 512], F32, tag="psk")
                    for cc in range(CC):
                        nc.tensor.matmul(psk, lhsT=wk_sb[:, cc, :], rhs=kvt[:, cc, :],
                                         start=(cc == 0), stop=(cc == CC - 1))
                    nc.scalar.copy(out=Kd_sb[:, t0:t0 + 512], in_=psk)

                    psq = psP.tile([128, 512], F32, tag="psq")
                    for cc in range(CC):
                        nc.tensor.matmul(psq, lhsT=wq_sb[:, cc, :], rhs=qt_[:, cc, :],
                                         start=(cc == 0), stop=(cc == CC - 1))
                    nc.scalar.copy(out=Qd_sb[:, t0:t0 + 512], in_=psq)

                    for s4 in range(4):
                        psv = psP.tile([128, 128], F32, tag="psv")
                        for cc in range(CC):
                            nc.tensor.matmul(psv, lhsT=kvt[:, cc, s4 * 128:(s4 + 1) * 128],
                                             rhs=wv_sb[:, cc, :],
                                             start=(cc == 0), stop=(cc == CC - 1))
                        ti = tt * 4 + s4
                        nc.vector.tensor_copy(out=V_sb[:, ti, 0:64], in_=psv[:, 0:64])
                        nc.vector.tensor_copy(out=V_sb[:, ti, 65:129], in_=psv[:, 64:128])

            a2a_in = dram.tile([N_CORES, DL, TSH], BF16)
            a2a_out = dram.tile([N_CORES, DL, TSH], BF16)

            # ---- attention ----
            with (
                tc.tile_pool(name="psS", bufs=2, space="PSUM") as psS,
                tc.tile_pool(name="psC", bufs=2, space="PSUM") as psC,
            ):
                for b in range(B):
                    for qh in range(2):            # q-tiles of 1024
                        q0 = b * NQ + qh * 1024
                        psc = [psC.tile([65, 1024], F32, tag="psc", name=f"psc{_h}") for _h in range(2)]
                        for kt in range(KT):
                            vt = b * KT + kt
                            k0 = b * NKV + kt * 128
                            pss = [psS.tile([128, 1024], F32, tag="pss", name=f"pss{_h}") for _h in range(2)]
                            for h in range(2):
                                hs = slice(h * 64, (h + 1) * 64)
                                for qx in range(2):
                                    nc.tensor.matmul(
                                        pss[h][:, qx * 512:(qx + 1) * 512],
                                        lhsT=Kd_sb[hs, k0:k0 + 128],
                                        rhs=Qd_sb[hs, q0 + qx * 512: q0 + (qx + 1) * 512],
                                        start=True, stop=True)
                            for h in range(2):
                                pr = probs_p.tile([128, 1024], BF16, tag="probs")
                                nc.scalar.activation(out=pr, in_=pss[h], func=Exp,
                                                     scale=SCALE)
                                for qx in range(2):
                                    nc.tensor.matmul(
                                        psc[h][:, qx * 512:(qx + 1) * 512],
                                        lhsT=V_sb[:, vt, h * 65:(h + 1) * 65],
                                        rhs=pr[:, qx * 512:(qx + 1) * 512],
                                        start=(kt == 0), stop=(kt == KT - 1))
                        for h in range(2):
                            recip = norm.tile([1, 1024], F32, tag="recip")
                            nc.vector.reciprocal(out=recip, in_=psc[h][64:65, :])
                            bc = norm.tile([64, 1024], F32, tag="bc")
                            nc.gpsimd.partition_broadcast(bc[:], recip[:])
                            ctxn = norm.tile([64, 1024], BF16, tag="ctxn")
                            nc.vector.tensor_mul(ctxn, psc[h][0:64, :], bc)
                            for qx in range(2):
                                j = q0 // TSH + qx
                                nc.sync.dma_start(
                                    out=a2a_in[j, h * 64:(h + 1) * 64, :],
                                    in_=ctxn[:, qx * 512:(qx + 1) * 512])

            if with_collective:
                nc.gpsimd.collective_compute(
                    "AllToAll", mybir.AluOpType.bypass,
                    replica_groups=[list(range(N_CORES))],
                    ins=[a2a_in.opt()], outs=[a2a_out.opt()])
            else:
                a2a_out = a2a_in  # timing-sim variant: skip collective

            # ---- output projection (local tokens only) ----
            with tc.tile_pool(name="psO", bufs=2, space="PSUM") as psO:
                ctxF = outp.tile([128, N_CORES, TSH], BF16)
                nc.sync.dma_start(out=ctxF, in_=a2a_out.rearrange("i p t -> p i t"))
                for m in range(TSH // 128):
                    ob = outp.tile([128, C], F32, tag="ob")
                    for half in range(2):
                        pso = psO.tile([128, 512], F32, tag="pso")
                        for i in range(N_CORES):
                            nc.tensor.matmul(
                                pso, lhsT=ctxF[:, i, m * 128:(m + 1) * 128],
                                rhs=wo_sb[:, i, half * 512:(half + 1) * 512],
                                start=(i == 0), stop=(i == N_CORES - 1))
                        nc.vector.tensor_add(ob[:, half * 512:(half + 1) * 512],
                                             pso, bias_sb[:, half * 512:(half + 1) * 512])
                    nc.sync.dma_start(out=out.ap()[m * 128:(m + 1) * 128, :], in_=ob)

    nc.compile()
    return nc


def _get_nc():
    global _NC_CACHE
    if _NC_CACHE is None:
        _NC_CACHE = _build()
    return _NC_CACHE


def prep_in_maps(query, key_value, w_q, w_kv, w_out, b_out):
    bf = ml_dtypes.bfloat16
    q2 = np.asarray(query, np.float32).reshape(T, C)
    kv2 = np.asarray(key_value, np.float32).reshape(T, C)
    qT = np.ascontiguousarray(q2.T).astype(bf)
    kvT = np.ascontiguousarray(kv2.T).astype(bf)
    wo = np.asarray(w_out, np.float32).astype(bf)
    bias = np.asarray(b_out, np.float32)

    in_maps = []
    for j in range(N_CORES):
        cs = slice(j * DL, (j + 1) * DL)
        in_maps.append({
            "qT": qT,
            "kvT": kvT,
            "wq": np.ascontiguousarray(np.asarray(w_q, np.float32)[:, cs]).astype(bf),
            "wk": np.ascontiguousarray(np.asarray(w_kv, np.float32)[:, cs]).astype(bf),
            "wv": np.ascontiguousarray(
                np.asarray(w_kv, np.float32)[:, C + j * DL: C + (j + 1) * DL]).astype(bf),
            "wo": wo,
            "bias": bias,
        })
    return in_maps


def kernel(query, key_value, w_q, w_kv, w_out, b_out):
    global _LAST_RESULTS
    in_maps = prep_in_maps(query, key_value, w_q, w_kv, w_out, b_out)
    nc = _get_nc()
    res = run_bass_kernel_spmd(nc, in_maps, core_ids=list(range(N_CORES)))
    _LAST_RESULTS = res
    full = np.concatenate([res.results[j]["out"] for j in range(N_CORES)], axis=0)
    return full.reshape(B, NQ, C)
